# revision 1
# baseline (speedup 1.0000x reference)
"""Trainium2 Bass kernel for nn_AttentionBlock (GroupNorm + single-head spatial
self-attention + residual) on 8 NeuronCores.

Sharding: data-parallel over batch (2) x sequence-parallel over the query
dimension (4 chunks of 1024 of the 4096 spatial tokens). Each core gets the
full image of its batch element, ROTATED so its query chunk sits at token 0
(GroupNorm stats, key/value sets and softmax sums are permutation-invariant
over tokens, so rotation lets all 8 cores run the identical SPMD program).

Per-core dataflow (channel-major [C on partitions] everywhere except v):
  phase 1: GroupNorm stats via bn_stats/bn_aggr per channel, group-combine via
           a tiny PE matmul with a 1/16 block indicator, broadcast back with a
           second indicator matmul -> per-channel Scale a[c] / Bias b[c].
  fold:    the GroupNorm affine shift never materializes: the scale a is one
           in-place per-partition multiply on each streamed x chunk (the f32r
           rounding producer), and the shift b becomes per-output-channel
           constants (qb = wq.b + bq etc.) computed with tiny N=2 PE matmuls;
           v's constant rides through softmax (attention rows sum to 1) and
           lands in the y epilogue constant yb = wp.(wv.b + bv) + bp.
  phase 2: stream raw x in 512-token chunks straight into the PE:
           k [C, 4096], vT [4096, C] (transposed layout so the later AV matmul
           needs no transposes), q [C, 1024] (first two chunks = query tokens).
  phase 3: per 512-query half: scores^T [j:128, i:512] = k_tile^T @ q (PSUM
           accum over C), exp on ScalarE straight out of PSUM (no max
           subtraction -- logits are O(5)), row-sums r via a ones-vector
           matmul, AV accum hattn[c, i] += vT_tile^T @ p with no transposes,
           softmax normalization deferred: 1/r via exp(-ln r) on ScalarE,
           broadcast with a K=1 matmul, folded into the PSUM->SBUF move;
           y = wp @ hattn + yb + x in a single fused DVE op per tile. The
           next half's first score/exp iterations are emitted into the
           softmax-tail window so the PE never idles across halves.

Matmuls run as float32r (fp32 storage, reduced-precision PE multiply at 4x
the fp32 matmul rate); the residual dominates the output so the attention
path has ~20x error dilution.
"""

import sys
from contextlib import ExitStack

if "/opt/trn_rl_repo" not in sys.path:
    sys.path.insert(0, "/opt/trn_rl_repo")

import numpy as np

import concourse.bass as bass  # noqa: F401  (import keeps bass registered)
import concourse.tile as tile
from concourse import bacc, mybir
from concourse.alu_op_type import AluOpType
from concourse.bass_utils import run_bass_kernel_spmd

F32 = mybir.dt.float32
F32R = mybir.dt.float32r
AF = mybir.ActivationFunctionType
OP = AluOpType

B, C, H, W = 2, 512, 64, 64
HW = H * W          # 4096 spatial tokens
P = 128             # partitions
CT = C // P         # 4 channel tiles
NCORES = 8
QN = HW // 4        # 1024 queries per core
CHW = 512           # token chunk width
NCH = HW // CHW     # 8 chunks
JT = HW // P        # 32 key tiles
EPS = 1e-6
SCALE = float(C) ** -0.5
GPT = P // 16       # 8 groups per channel tile

MDT = F32R


def _build_body(nc, tc, ctx, d):
    xb_d = d["xb"]
    wT_d = {n: d[n] for n in ("wqT", "wkT", "wvT", "wpT")}
    y_d = d["y"]

    cpool = ctx.enter_context(tc.tile_pool(name="const", bufs=1))
    ppool = ctx.enter_context(tc.tile_pool(name="persist", bufs=1))
    spool = ctx.enter_context(tc.tile_pool(name="stream", bufs=2))
    smpool = ctx.enter_context(tc.tile_pool(name="small", bufs=1))
    qpool = ctx.enter_context(tc.tile_pool(name="psum", bufs=2, space="PSUM"))

    # ---- phase 1: GroupNorm statistics (4 c-tile chains interleaved) ----
    ind = cpool.tile([P, GPT], F32, tag="ind")
    nc.scalar.dma_start(ind[:], d["ind"][:])
    gps = qpool.tile([GPT, 2 * CT], F32, tag="pa")
    sts = [smpool.tile([P, NCH, 6], F32, tag="st", bufs=CT, name=f"st{t}")
           for t in range(CT)]
    dma_engines = [nc.gpsimd, nc.scalar, nc.sync]
    # stats x streams into the (still idle) k_sb slots: all 16 transfers can
    # be in flight at once, and the slots hand over to k_sb afterward
    xstat = [ppool.tile([P, HW], F32, tag=f"k{t}", name=f"xstat{t}")
             for t in range(CT)]
    for half in range(NCH // 2):
        for t in range(CT):
            eng = dma_engines[(half * CT + t) % 3]
            eng.dma_start(xstat[t][:, half * 2 * CHW:(half + 1) * 2 * CHW],
                          xb_d[half, t])
    for ch in range(NCH):
        for t in range(CT):
            nc.vector.bn_stats(sts[t][:, ch, :],
                               xstat[t][:, ch * CHW:(ch + 1) * CHW])
    # ---- small constants (emitted after the x stream so the stats DMAs
    # are first in every engine's trigger queue) ----
    chv = []
    for t in range(CT):
        v = cpool.tile([P, 6], F32, tag=f"chv{t}", name=f"chv{t}")
        nc.gpsimd.dma_start(v[:], d["chv"][t])
        chv.append(v)
    # chv columns: gamma, beta, bq, bk, bv, bp
    indT = cpool.tile([GPT, P], F32, tag="indT")
    nc.gpsimd.dma_start(indT[:], d["indT"][:])
    ones_col = cpool.tile([P, 1], MDT, tag="onesc")
    nc.gpsimd.dma_start(ones_col[:], d["ones_col"][:])
    ones_r32 = smpool.tile([1, P], F32, tag="onesr32")
    nc.vector.memset(ones_r32[:], 1.0)
    epst = smpool.tile([GPT, 1], F32, tag="eps")
    nc.vector.memset(epst[:], EPS)

    for t in range(CT):
        mv = smpool.tile([P, 2], F32, tag="mv", bufs=1)
        nc.vector.bn_aggr(mv[:], sts[t][:])
        sq = smpool.tile([P, 1], F32, tag="sq", bufs=1)
        nc.vector.tensor_tensor(sq[:], mv[:, 0:1], mv[:, 0:1], op=OP.mult)
        s2 = smpool.tile([P, 2], F32, tag="s2", bufs=1)
        nc.vector.tensor_copy(s2[:, 0:1], mv[:, 0:1])
        nc.vector.tensor_tensor(s2[:, 1:2], sq[:], mv[:, 1:2], op=OP.add)
        nc.tensor.matmul(gps[:, 2 * t:2 * t + 2], ind[:], s2[:],
                         start=True, stop=True)

    gst = smpool.tile([GPT, 2 * CT], F32, tag="gst")
    nc.vector.tensor_copy(gst[:], gps[:])
    g3 = gst.rearrange("p (t two) -> p t two", two=2)
    msq = smpool.tile([GPT, CT], F32, tag="msq")
    nc.vector.tensor_tensor(msq[:], g3[:, :, 0], g3[:, :, 0], op=OP.mult)
    varg = smpool.tile([GPT, CT], F32, tag="varg")
    nc.vector.tensor_tensor(varg[:], g3[:, :, 1], msq[:], op=OP.subtract)
    stdg = smpool.tile([GPT, CT], F32, tag="stdg")
    nc.scalar.activation(stdg[:], varg[:], AF.Sqrt, bias=epst[:])
    # interleave (mu_t, rstd_t) columns and broadcast all groups->channels
    # with a single [K=8, M=128, N=8] indicator matmul
    mr = smpool.tile([GPT, 2 * CT], F32, tag="mr")
    mr3 = mr.rearrange("p (t two) -> p t two", two=2)
    nc.vector.tensor_copy(mr3[:, :, 0], g3[:, :, 0])
    nc.vector.reciprocal(mr3[:, :, 1], stdg[:])
    cba = qpool.tile([P, 2 * CT], F32, tag="pa")
    nc.tensor.matmul(cba[:], indT[:], mr[:], start=True, stop=True)
    cb = smpool.tile([P, 2 * CT], F32, tag="cb")
    nc.vector.tensor_copy(cb[:], cba[:])

    # per-channel Scale a (col 0) / Bias b (col 1); bvec = f32r copy of b
    sbts, bvec = [], []
    for t in range(CT):
        sbt = ppool.tile([P, 2], F32, tag=f"sb{t}")
        nc.vector.tensor_tensor(sbt[:, 0:1], cb[:, 2 * t + 1:2 * t + 2],
                                chv[t][:, 0:1], op=OP.mult)
        tmpb = smpool.tile([P, 1], F32, tag="tmpb", bufs=1)
        nc.vector.tensor_tensor(tmpb[:], cb[:, 2 * t:2 * t + 1], sbt[:, 0:1],
                                op=OP.mult)
        nc.vector.tensor_tensor(sbt[:, 1:2], chv[t][:, 1:2], tmpb[:],
                                op=OP.subtract)
        bv_ = ppool.tile([P, 2], MDT, tag=f"bvec{t}", name=f"bvec{t}")
        nc.vector.tensor_copy(bv_[:, 0:1], sbt[:, 1:2])
        nc.vector.tensor_copy(bv_[:, 1:2], sbt[:, 1:2])
        sbts.append(sbt)
        bvec.append(bv_)

    # ---- bulk constants: projection weights, in consumption order (k is
    # needed first by the bias matmuls and first projections), spread across
    # the three DMA-capable engines ----
    wts = {}
    for wi, name in enumerate(("wkT", "wvT", "wqT")):
        wts[name] = []
        for t in range(CT):
            tag = f"wkp{t}" if name == "wkT" else f"{name}{t}"
            w = cpool.tile([P, C], MDT, tag=tag, name=f"{name}{t}")
            dma_engines[(wi * CT + t) % 3].dma_start(w[:], wT_d[name][t])
            wts[name].append(w)

    # ---- bias-term constants from ORIGINAL weights (tiny N=1 matmuls) ----
    #   qb[o] = sum_c wq[o,c] b[c] + bq    (per-partition add at the q copy)
    #   kb[o] = likewise with bk
    #   vbt[c] = sum_cin wv[c,cin] b[cin] + bv   (rides softmax into yb)
    #   yb[o] = sum_c wp[o,c] vbt[c] + bp        (y epilogue constant)
    def bias_contract(wtiles, rhs_tiles, outdt, addcol, tagp, two_col=False):
        outs = []
        for ot in range(CT):
            pb = qpool.tile([P, 2], F32, tag="pa")
            for t in range(CT):
                nc.tensor.matmul(pb[:], wtiles[t][:, ot * P:(ot + 1) * P],
                                 rhs_tiles[t][:, 0:2], start=(t == 0),
                                 stop=(t == CT - 1))
            w = 2 if two_col else 1
            ob = ppool.tile([P, w], outdt, tag=f"{tagp}{ot}", name=f"{tagp}{ot}")
            nc.vector.tensor_scalar(ob[:], pb[:, 0:w],
                                    chv[ot][:, addcol:addcol + 1],
                                    None, OP.add)
            outs.append(ob)
        return outs

    kb = bias_contract(wts["wkT"], bvec, F32, 3, "kb")
    vbt = bias_contract(wts["wvT"], bvec, MDT, 4, "vbt", two_col=True)
    qb = bias_contract(wts["wqT"], bvec, F32, 2, "qb")



    # ---- persistent attention operands ----
    k_sb = [ppool.tile([P, HW], MDT, tag=f"k{t}", name=f"k{t}") for t in range(CT)]
    q_sb = [ppool.tile([P, QN], MDT, tag=f"q{t}", name=f"q{t}") for t in range(CT)]
    vT_sb = [ppool.tile([P, C], MDT, tag=f"vT{j}", name=f"vT{j}") for j in range(JT)]

    # ---- phase 2: q/k/v projections, streamed over raw x token chunks ----
    for ch in range(NCH):
        sl = slice(ch * CHW, (ch + 1) * CHW)
        xts = []
        for t in range(CT):
            xt = spool.tile([P, CHW], F32, tag="sx", bufs=3)
            eng = nc.sync if (ch + t) % 2 == 0 else nc.gpsimd
            eng.dma_start(xt[:], xb_d[ch // 2, t,
                                      :, (ch % 2) * CHW:(ch % 2 + 1) * CHW])
            # GroupNorm scale (the shift rides in kb/qb/yb); rounds to f32r
            xs = spool.tile([P, CHW], MDT, tag=f"hx{t}", bufs=2)
            nc.vector.tensor_scalar_mul(xs[:], xt[:], sbts[t][:, 0:1])
            xts.append(xs)
        for ot in range(CT):
            pk = qpool.tile([P, CHW], F32, tag="pa")
            for t in range(CT):
                nc.tensor.matmul(pk[:], wts["wkT"][t][:, ot * P:(ot + 1) * P],
                                 xts[t][:], start=(t == 0), stop=(t == CT - 1))
            nc.vector.tensor_scalar(k_sb[ot][:, sl], pk[:], kb[ot][:],
                                    None, OP.add)
        for nt in range(CT):
            pv = qpool.tile([P, CHW], F32, tag="pa")
            for t in range(CT):
                nc.tensor.matmul(pv[:], xts[t][:, nt * P:(nt + 1) * P],
                                 wts["wvT"][t][:], start=(t == 0),
                                 stop=(t == CT - 1))
            nc.scalar.copy(vT_sb[ch * CT + nt][:], pv[:])
        if ch * CHW < QN:
            for ot in range(CT):
                pq = qpool.tile([P, CHW], F32, tag="pa")
                for t in range(CT):
                    nc.tensor.matmul(pq[:], wts["wqT"][t][:, ot * P:(ot + 1) * P],
                                     xts[t][:], start=(t == 0),
                                     stop=(t == CT - 1))
                nc.vector.tensor_scalar(q_sb[ot][:, sl], pq[:], qb[ot][:],
                                        None, OP.add)

    # ---- phase 3: attention, per query half ----
    # wpT reuses wkT's slots (k_sb is materialized, wkT is dead)
    wts["wpT"] = []
    for t in range(CT):
        w = cpool.tile([P, C], MDT, tag=f"wkp{t}", name=f"wpT{t}")
        nc.sync.dma_start(w[:], wT_d["wpT"][t])
        wts["wpT"].append(w)
    yb = bias_contract(wts["wpT"], vbt, F32, 5, "yb")

    def mk_pr():
        return qpool.tile([1, CHW], F32, tag="pr", bufs=1, name="pr")

    def mk_po():
        return [qpool.tile([P, CHW], F32, tag=f"po{t}", name=f"po{t}", bufs=1)
                for t in range(CT)]

    def score_exp(pr, ih, j):
        isl = slice(ih * CHW, (ih + 1) * CHW)
        ps_ = qpool.tile([P, CHW], F32, tag="pa", name="ps")
        for t in range(CT):
            nc.tensor.matmul(ps_[:], k_sb[t][:, j * P:(j + 1) * P],
                             q_sb[t][:, isl], start=(t == 0), stop=(t == CT - 1))
        pT = spool.tile([P, CHW], MDT, tag="pT", bufs=4, name="pT")
        nc.scalar.activation(pT[:], ps_[:], AF.Exp, scale=SCALE)
        nc.tensor.matmul(pr[:], ones_col[:], pT[:],
                         start=(j == 0), stop=(j == JT - 1))
        return pT

    def av(po, j, pT):
        for t in range(CT):
            nc.tensor.matmul(po[t][:], vT_sb[j][:, t * P:(t + 1) * P],
                             pT[:], start=(j == 0), stop=(j == JT - 1))

    def tail_and_y(pr, po, ih):
        isl = slice(ih * CHW, (ih + 1) * CHW)
        rsb = spool.tile([1, CHW], F32, tag="sx", bufs=3)
        nc.vector.tensor_copy(rsb[:], pr[:])
        # 1/r via exp(-ln(r)) on ScalarE, in place: faster than DVE's
        # iterative reciprocal and only one stream-pool slot
        nc.scalar.activation(rsb[:], rsb[:], AF.Ln)
        nc.scalar.activation(rsb[:], rsb[:], AF.Exp, scale=-1.0)
        prb = qpool.tile([P, CHW], F32, tag="pa")
        nc.tensor.matmul(prb[:], ones_r32[:], rsb[:], start=True, stop=True)
        rb = spool.tile([P, CHW], F32, tag="sx", bufs=3)
        nc.vector.tensor_copy(rb[:], prb[:])
        has = []
        for t in range(CT):
            ha = spool.tile([P, CHW], MDT, tag=f"hx{t}", bufs=2)
            nc.vector.tensor_tensor(ha[:], po[t][:], rb[:], op=OP.mult)
            has.append(ha)
        for ot in range(CT):
            py = qpool.tile([P, CHW], F32, tag="pa")
            for t in range(CT):
                nc.tensor.matmul(py[:], wts["wpT"][t][:, ot * P:(ot + 1) * P],
                                 has[t][:], start=(t == 0), stop=(t == CT - 1))
            xr = spool.tile([P, CHW], F32, tag="sx", bufs=3)
            nc.sync.dma_start(xr[:], xb_d[0, ot, :, isl])
            yt = spool.tile([P, CHW], F32, tag="pT", bufs=4, name="yt")
            nc.vector.scalar_tensor_tensor(yt[:], py[:], yb[ot][:, 0:1],
                                           xr[:], op0=OP.add, op1=OP.add)
            nc.gpsimd.dma_start(y_d[ot, :, isl], yt[:])

    KPRE = 4  # ih1 score/exp iterations prefetched into ih0's softmax tail
    pr0 = mk_pr()
    po0 = mk_po()
    for j in range(JT):
        av(po0, j, score_exp(pr0, 0, j))
    pr1 = mk_pr()
    pre = [score_exp(pr1, 1, j) for j in range(KPRE)]
    tail_and_y(pr0, po0, 0)
    po1 = mk_po()
    for j in range(JT):
        pT = pre[j] if j < KPRE else score_exp(pr1, 1, j)
        av(po1, j, pT)
    tail_and_y(pr1, po1, 1)


def build_module():
    nc = bacc.Bacc("TRN2", target_bir_lowering=False, debug=False,
                   num_devices=NCORES)
    d = {
        "xb": nc.dram_tensor("xb", [NCH // 2, CT, P, 2 * CHW], F32,
                             kind="ExternalInput").ap(),
        "wqT": nc.dram_tensor("wqT", [CT, P, C], MDT, kind="ExternalInput").ap(),
        "wkT": nc.dram_tensor("wkT", [CT, P, C], MDT, kind="ExternalInput").ap(),
        "wvT": nc.dram_tensor("wvT", [CT, P, C], MDT, kind="ExternalInput").ap(),
        "wpT": nc.dram_tensor("wpT", [CT, P, C], MDT, kind="ExternalInput").ap(),
        "chv": nc.dram_tensor("chv", [CT, P, 6], F32, kind="ExternalInput").ap(),
        "ind": nc.dram_tensor("ind", [P, GPT], F32, kind="ExternalInput").ap(),
        "indT": nc.dram_tensor("indT", [GPT, P], F32, kind="ExternalInput").ap(),
        "ones_col": nc.dram_tensor("ones_col", [P, 1], MDT,
                                   kind="ExternalInput").ap(),
        "y": nc.dram_tensor("y", [CT, P, QN], F32, kind="ExternalOutput").ap(),
    }
    with tile.TileContext(nc) as tc, ExitStack() as ctx:
        _build_body(nc, tc, ctx, d)
    nc.compile()
    return nc


_CACHE = {}


def _get_nc():
    if "nc" not in _CACHE:
        _CACHE["nc"] = build_module()
    return _CACHE["nc"]


def _shared_inputs(gamma, beta, wq, bq, wk, bk, wv, bv, wp, bp):
    def wT(w):
        return np.ascontiguousarray(np.asarray(w, np.float32).T).reshape(CT, P, C)

    ind = np.zeros((P, GPT), np.float32)
    for i in range(P):
        ind[i, i // 16] = 1.0 / 16.0
    indT = np.zeros((GPT, P), np.float32)
    for i in range(P):
        indT[i // 16, i] = 1.0
    chv = np.stack([np.asarray(a, np.float32)
                    for a in (gamma, beta, bq, bk, bv, bp)],
                   axis=1).reshape(CT, P, 6)
    return {
        "wqT": wT(wq), "wkT": wT(wk), "wvT": wT(wv), "wpT": wT(wp),
        "chv": np.ascontiguousarray(chv),
        "ind": ind, "indT": indT,
        "ones_col": np.ones((P, 1), np.float32),
    }


def make_in_maps(x, gamma, beta, wq, bq, wk, bk, wv, bv, wp, bp):
    shared = _shared_inputs(gamma, beta, wq, bq, wk, bk, wv, bv, wp, bp)
    xf = np.asarray(x, np.float32).reshape(B, C, HW)
    in_maps = []
    for core in range(NCORES):
        b, qc = divmod(core, NCORES // B)
        xb = np.roll(xf[b], -qc * QN, axis=1)          # [C, HW]
        xt = xb.reshape(CT, P, NCH // 2, 2 * CHW).transpose(2, 0, 1, 3)
        m = dict(shared)
        m["xb"] = np.ascontiguousarray(xt)
        in_maps.append(m)
    return in_maps


def assemble_output(results):
    out = np.empty((B, C, HW), np.float32)
    for core in range(NCORES):
        b, qc = divmod(core, NCORES // B)
        y = np.asarray(results[core]["y"]).reshape(C, QN)
        out[b, :, qc * QN:(qc + 1) * QN] = y
    return out.reshape(B, C, H, W)


def kernel(x, gamma, beta, wq, bq, wk, bk, wv, bv, wp, bp):
    nc = _get_nc()
    in_maps = make_in_maps(x, gamma, beta, wq, bq, wk, bk, wv, bv, wp, bp)
    res = run_bass_kernel_spmd(nc, in_maps, list(range(NCORES)))
    return assemble_output(res.results)



# revision 9
# speedup vs baseline: 1.4535x; 1.4535x over previous
"""Trainium2 Bass kernel for nn_AttentionBlock (GroupNorm + single-head spatial
self-attention + residual) on 8 NeuronCores.

Sharding: data-parallel over batch (2) x sequence-parallel over the query
dimension (4 chunks of 1024 of the 4096 spatial tokens). Each core gets the
full image of its batch element, ROTATED so its query chunk sits at token 0.

fp8 (e4m3) DoubleRow redesign vs the f32r baseline:
  - Host pre-casts x to fp8 pairs (matmul operand) and fp16 (residual, query
    chunk only); weights wq/wk/wv go as bf16 of 8*w^T, wp stays f32r.
  - GroupNorm stats run on the fp8 x (bn_stats accepts fp8; the
    self-referential normalization makes quantized stats consistent).
  - The GroupNorm scale a folds into the weights on-chip (bf16 * a -> fp8
    pairs); the shift b becomes per-output-channel constants computed with
    fp8 DoubleRow matmuls against a 64*b/a column pair.
  - All large matmuls (k/v/q projections, scores, AV, row-sums) run as fp8
    DoubleRow (K=256 per instruction, 2-4x the f32r rate). Only the output
    projection stays f32r for accuracy.
  - exp logits are shifted by -2.5 to center the p distribution in e4m3's
    normal range (max logit ~7.3 -> p_max ~120 < 240); the shift cancels in
    the softmax normalization.
Numpy simulation of this exact scheme: rel err ~5.7e-3 (tolerance 2e-2).
"""

import sys
from contextlib import ExitStack

if "/opt/trn_rl_repo" not in sys.path:
    sys.path.insert(0, "/opt/trn_rl_repo")

import numpy as np
import ml_dtypes

import concourse.bass as bass  # noqa: F401  (import keeps bass registered)
import concourse.tile as tile
from concourse import bacc, mybir
from concourse.alu_op_type import AluOpType
from concourse.bass_utils import run_bass_kernel_spmd

F32 = mybir.dt.float32
F32R = mybir.dt.float32r
F16 = mybir.dt.float16
BF16 = mybir.dt.bfloat16
FP8 = mybir.dt.float8e4
AF = mybir.ActivationFunctionType
OP = AluOpType
DR = mybir.MatmulPerfMode.DoubleRow

B, C, H, W = 2, 512, 64, 64
HW = H * W          # 4096 spatial tokens
P = 128             # partitions
CT = C // P         # 4 channel tiles
NP = CT // 2        # 2 channel-pair tiles
NCORES = 8
QN = HW // 4        # 1024 queries per core
CHW = 512           # token chunk width
NCH = HW // CHW     # 8 chunks
JT = HW // P        # 32 key tiles
JP = JT // 2        # 16 key-tile pairs
EPS = 1e-6
SCALE = float(C) ** -0.5
SHIFT = 2.5         # exp(s*SCALE - SHIFT); cancels in softmax normalization
BA = 64.0           # b/a column pre-scale (keeps fp8 values in normal range)
GPT = P // 16       # 8 groups per channel tile


def _build_body(nc, tc, ctx, d):
    cpool = ctx.enter_context(tc.tile_pool(name="const", bufs=1))
    ppool = ctx.enter_context(tc.tile_pool(name="persist", bufs=1))
    spool = ctx.enter_context(tc.tile_pool(name="stream", bufs=2))
    smpool = ctx.enter_context(tc.tile_pool(name="small", bufs=1))
    qpool = ctx.enter_context(tc.tile_pool(name="psum", bufs=2, space="PSUM"))

    # ---- DMA: fp8 x pairs (8 quarter transfers on sync+gpsimd), small
    # consts + bf16 weights on scalar; wpT/x16q trail on sync/gpsimd ----
    xp = [ppool.tile([P, 2, HW], FP8, tag=f"xp{u}", name=f"xp{u}")
          for u in range(NP)]
    QTR = HW // 4
    for qt in range(4):
        sl = slice(qt * QTR, (qt + 1) * QTR)
        nc.sync.dma_start(xp[0][:, :, sl], d["x8p"][0, :, :, sl])
        nc.gpsimd.dma_start(xp[1][:, :, sl], d["x8p"][1, :, :, sl])

    cblob = cpool.tile([P, 32], F32, tag="cblob")
    nc.scalar.dma_start(cblob[:], d["cblob"][:])
    chv = [cblob[:, 6 * t:6 * t + 6] for t in range(CT)]
    ind = cblob[:, 24:32]
    indT = cpool.tile([GPT, P], F32, tag="indT")
    nc.scalar.dma_start(indT[:], d["indT"][:])
    # pair-dim stride must be a multiple of 16 for dual-fp8 LDWEIGHTS/MATMUL,
    # so the tiny pair operands are padded to [P, 2, 16]
    ones2 = cpool.tile([P, 2, 16], FP8, tag="ones2")
    nc.scalar.dma_start(ones2[:], d["ones2"][:])
    wbf = {}
    for name in ("wkbf", "wvbf", "wqbf"):
        wbf[name] = []
        for t in range(CT):
            w = cpool.tile([P, C], BF16, tag=f"{name}{t}", name=f"{name}{t}")
            nc.scalar.dma_start(w[:], d[name][t])
            wbf[name].append(w)
    wpT = []
    for t in range(CT):
        w = cpool.tile([P, C], F32R, tag=f"wpT{t}", name=f"wpT{t}")
        nc.sync.dma_start(w[:], d["wpT"][t])
        wpT.append(w)
    x16 = []
    for t in range(CT):
        xt = ppool.tile([P, QN], F16, tag=f"x16{t}", name=f"x16{t}")
        nc.gpsimd.dma_start(xt[:], d["x16q"][t])
        x16.append(xt)

    # ---- phase 1: GroupNorm statistics from the fp8 x ----
    sts = [smpool.tile([P, NCH, 6], F32, tag="st", bufs=CT, name=f"st{t}")
           for t in range(CT)]
    for qt in range(4):
        for u in range(NP):
            for i in range(2):
                for ch in (2 * qt, 2 * qt + 1):
                    nc.vector.bn_stats(
                        sts[2 * u + i][:, ch, :],
                        xp[u][:, i, ch * CHW:(ch + 1) * CHW])

    ones_r32 = smpool.tile([1, P], F32, tag="onesr32")
    nc.vector.memset(ones_r32[:], 1.0)
    epst = smpool.tile([GPT, 1], F32, tag="eps")
    nc.vector.memset(epst[:], EPS)
    nshift = smpool.tile([P, 1], F32, tag="nshift")
    nc.vector.memset(nshift[:], -SHIFT)

    gps = qpool.tile([GPT, 2 * CT], F32, tag="pa")
    for t in range(CT):
        mv = smpool.tile([P, 2], F32, tag="mv", bufs=1)
        nc.vector.bn_aggr(mv[:], sts[t][:])
        sq = smpool.tile([P, 1], F32, tag="sq", bufs=1)
        nc.vector.tensor_tensor(sq[:], mv[:, 0:1], mv[:, 0:1], op=OP.mult)
        s2 = smpool.tile([P, 2], F32, tag="s2", bufs=1)
        nc.vector.tensor_copy(s2[:, 0:1], mv[:, 0:1])
        nc.vector.tensor_tensor(s2[:, 1:2], sq[:], mv[:, 1:2], op=OP.add)
        nc.tensor.matmul(gps[:, 2 * t:2 * t + 2], ind[:], s2[:],
                         start=True, stop=True)

    gst = smpool.tile([GPT, 2 * CT], F32, tag="gst")
    nc.vector.tensor_copy(gst[:], gps[:])
    g3 = gst.rearrange("p (t two) -> p t two", two=2)
    msq = smpool.tile([GPT, CT], F32, tag="msq")
    nc.vector.tensor_tensor(msq[:], g3[:, :, 0], g3[:, :, 0], op=OP.mult)
    varg = smpool.tile([GPT, CT], F32, tag="varg")
    nc.vector.tensor_tensor(varg[:], g3[:, :, 1], msq[:], op=OP.subtract)
    stdg = smpool.tile([GPT, CT], F32, tag="stdg")
    nc.scalar.activation(stdg[:], varg[:], AF.Sqrt, bias=epst[:])
    mr = smpool.tile([GPT, 2 * CT], F32, tag="mr")
    mr3 = mr.rearrange("p (t two) -> p t two", two=2)
    nc.vector.tensor_copy(mr3[:, :, 0], g3[:, :, 0])
    nc.vector.reciprocal(mr3[:, :, 1], stdg[:])
    cba = qpool.tile([P, 2 * CT], F32, tag="pa")
    nc.tensor.matmul(cba[:], indT[:], mr[:], start=True, stop=True)
    cb = smpool.tile([P, 2 * CT], F32, tag="cb")
    nc.vector.tensor_copy(cb[:], cba[:])

    # per-channel Scale a (col 0) / Bias b (col 1); bacol = fp8 of 64*b/a
    sbts = []
    bacol = [ppool.tile([P, 2, 16], FP8, tag=f"bac{u}", name=f"bac{u}")
             for u in range(NP)]
    for t in range(CT):
        sbt = ppool.tile([P, 2], F32, tag=f"sb{t}")
        nc.vector.tensor_tensor(sbt[:, 0:1], cb[:, 2 * t + 1:2 * t + 2],
                                chv[t][:, 0:1], op=OP.mult)
        tmpb = smpool.tile([P, 1], F32, tag="tmpb", bufs=1)
        nc.vector.tensor_tensor(tmpb[:], cb[:, 2 * t:2 * t + 1], sbt[:, 0:1],
                                op=OP.mult)
        nc.vector.tensor_tensor(sbt[:, 1:2], chv[t][:, 1:2], tmpb[:],
                                op=OP.subtract)
        ra = smpool.tile([P, 1], F32, tag="ra", bufs=1)
        nc.vector.reciprocal(ra[:], sbt[:, 0:1])
        bav = smpool.tile([P, 1], F32, tag="bav", bufs=1)
        nc.vector.tensor_tensor(bav[:], sbt[:, 1:2], ra[:], op=OP.mult)
        for col in range(2):
            nc.vector.tensor_scalar(bacol[t // 2][:, t % 2, col:col + 1],
                                    bav[:], BA, None, OP.mult)
        sbts.append(sbt)

    # ---- folds: fp8 weight pairs = bf16(8*w^T) * a ----
    w8 = {}
    for name, src, eng in (("k", "wkbf", nc.vector), ("v", "wvbf", nc.scalar),
                           ("q", "wqbf", nc.vector)):
        w8[name] = []
        for u in range(NP):
            w = cpool.tile([P, 2, C], FP8, tag=f"w8{name}{u}",
                           name=f"w8{name}{u}")
            for i in range(2):
                if eng is nc.scalar:
                    nc.scalar.activation(w[:, i, :], wbf[src][2 * u + i][:],
                                         AF.Copy, scale=sbts[2 * u + i][:, 0:1])
                else:
                    nc.vector.tensor_scalar_mul(w[:, i, :],
                                                wbf[src][2 * u + i][:],
                                                sbts[2 * u + i][:, 0:1])
            w8[name].append(w)

    # ---- bias constants via fp8 DoubleRow against the 64*b/a column ----
    #   out = sum_c 8*w[o,c]*a_c * 64*b_c/a_c = 512 * (w @ b)[o]
    def bias_dr(wkey, addcol, outdt, tagp):
        outs = []
        for ot in range(CT):
            pb = qpool.tile([P, 2], F32, tag="pa")
            for u in range(NP):
                nc.tensor.matmul(pb[:], w8[wkey][u][:, :, ot * P:(ot + 1) * P],
                                 bacol[u][:, :, 0:2], start=(u == 0),
                                 stop=(u == NP - 1), perf_mode=DR)
            w = 2 if outdt is F32R else 1
            ob = ppool.tile([P, w], outdt, tag=f"{tagp}{ot}", name=f"{tagp}{ot}")
            nc.vector.tensor_scalar(ob[:], pb[:, 0:w], 1.0 / (8.0 * BA),
                                    chv[ot][:, addcol:addcol + 1],
                                    OP.mult, OP.add)
            outs.append(ob)
        return outs

    kb = bias_dr("k", 3, F32, "kb")
    qb = bias_dr("q", 2, F32, "qb")

    # ---- persistent attention operands (fp8 pair layouts) ----
    k8p = [ppool.tile([P, 2, HW], FP8, tag=f"k8p{u}", name=f"k8p{u}")
           for u in range(NP)]
    q8p = [ppool.tile([P, 2, QN], FP8, tag=f"q8p{u}", name=f"q8p{u}")
           for u in range(NP)]
    vT8 = [ppool.tile([P, 2, C], FP8, tag=f"vT8{m}", name=f"vT8{m}")
           for m in range(JP)]

    # ---- phase 2: q/k/v projections, fp8 DoubleRow over token chunks ----
    for ch in range(NCH):
        sl = slice(ch * CHW, (ch + 1) * CHW)
        for ot in range(CT):
            pk = qpool.tile([P, CHW], F32, tag="pa")
            for u in range(NP):
                nc.tensor.matmul(pk[:], w8["k"][u][:, :, ot * P:(ot + 1) * P],
                                 xp[u][:, :, sl], start=(u == 0),
                                 stop=(u == NP - 1), perf_mode=DR)
            nc.vector.tensor_scalar(k8p[ot // 2][:, ot % 2, sl], pk[:],
                                    0.125, kb[ot][:], OP.mult, OP.add)
        for nt in range(CT):
            tok = slice(ch * CHW + nt * P, ch * CHW + (nt + 1) * P)
            pv = qpool.tile([P, C], F32, tag="pa")
            for u in range(NP):
                nc.tensor.matmul(pv[:], xp[u][:, :, tok], w8["v"][u][:],
                                 start=(u == 0), stop=(u == NP - 1),
                                 perf_mode=DR)
            j = ch * CT + nt
            nc.scalar.activation(vT8[j // 2][:, j % 2, :], pv[:],
                                 AF.Copy, scale=0.125)
        if ch < 2:
            for ot in range(CT):
                pq = qpool.tile([P, CHW], F32, tag="pa")
                for u in range(NP):
                    nc.tensor.matmul(pq[:],
                                     w8["q"][u][:, :, ot * P:(ot + 1) * P],
                                     xp[u][:, :, sl], start=(u == 0),
                                     stop=(u == NP - 1), perf_mode=DR)
                nc.vector.tensor_scalar(q8p[ot // 2][:, ot % 2, sl], pq[:],
                                        0.125, qb[ot][:], OP.mult, OP.add)

    # ---- phase 3: attention, per query half ----
    vbt = bias_dr("v", 4, F32R, "vbt")
    yb = []
    for ot in range(CT):
        pb = qpool.tile([P, 2], F32, tag="pa")
        for t in range(CT):
            nc.tensor.matmul(pb[:], wpT[t][:, ot * P:(ot + 1) * P],
                             vbt[t][:, 0:2], start=(t == 0), stop=(t == CT - 1))
        ob = ppool.tile([P, 1], F32, tag=f"yb{ot}", name=f"yb{ot}")
        nc.vector.tensor_scalar(ob[:], pb[:, 0:1], chv[ot][:, 5:6],
                                None, OP.add)
        yb.append(ob)

    def mk_pr():
        return qpool.tile([1, CHW], F32, tag="pr", bufs=2, name="pr")

    def mk_po():
        return [qpool.tile([P, CHW], F32, tag=f"po{t}", name=f"po{t}", bufs=1)
                for t in range(CT)]

    def score_exp_pair(pr, ih, jp):
        isl = slice(ih * CHW, (ih + 1) * CHW)
        pT2 = spool.tile([P, 2, CHW], FP8, tag="pT", bufs=4, name="pT")
        for jj in range(2):
            j = 2 * jp + jj
            ps_ = qpool.tile([P, CHW], F32, tag="pa", name="ps")
            for u in range(NP):
                nc.tensor.matmul(ps_[:], k8p[u][:, :, j * P:(j + 1) * P],
                                 q8p[u][:, :, isl], start=(u == 0),
                                 stop=(u == NP - 1), perf_mode=DR)
            nc.scalar.activation(pT2[:, jj, :], ps_[:], AF.Exp,
                                 scale=SCALE, bias=nshift[:])
        nc.tensor.matmul(pr[:], ones2[:, :, 0:1], pT2[:],
                         start=(jp == 0), stop=(jp == JP - 1), perf_mode=DR)
        return pT2

    def av_pair(po, jp, pT2):
        for t in range(CT):
            nc.tensor.matmul(po[t][:], vT8[jp][:, :, t * P:(t + 1) * P],
                             pT2[:], start=(jp == 0), stop=(jp == JP - 1),
                             perf_mode=DR)

    def tail_and_y(pr, po, ih, mid=None):
        isl = slice(ih * CHW, (ih + 1) * CHW)
        rsb = spool.tile([1, CHW], F32, tag="sx", bufs=3)
        nc.vector.tensor_copy(rsb[:], pr[:])
        # 1/r via exp(-ln(r)) on ScalarE
        nc.scalar.activation(rsb[:], rsb[:], AF.Ln)
        nc.scalar.activation(rsb[:], rsb[:], AF.Exp, scale=-1.0)
        prb = qpool.tile([P, CHW], F32, tag="pa")
        nc.tensor.matmul(prb[:], ones_r32[:], rsb[:], start=True, stop=True)
        rb = spool.tile([P, CHW], F32, tag="sx", bufs=3)
        nc.vector.tensor_copy(rb[:], prb[:])
        has = []
        for t in range(CT):
            ha = spool.tile([P, CHW], F32R, tag=f"hx{t}", bufs=2)
            nc.vector.tensor_tensor(ha[:], po[t][:], rb[:], op=OP.mult)
            has.append(ha)
        pre = mid() if mid is not None else []
        for ot in range(CT):
            py = qpool.tile([P, CHW], F32, tag="pa")
            for t in range(CT):
                nc.tensor.matmul(py[:], wpT[t][:, ot * P:(ot + 1) * P],
                                 has[t][:], start=(t == 0), stop=(t == CT - 1))
            yt = spool.tile([P, CHW], F32, tag="yt", bufs=2, name="yt")
            nc.vector.scalar_tensor_tensor(yt[:], py[:], yb[ot][:, 0:1],
                                           x16[ot][:, isl], op0=OP.add,
                                           op1=OP.add)
            nc.gpsimd.dma_start(d["y"][ot, :, isl], yt[:])
        return pre

    KPRE = 3  # next-half score/exp pairs prefetched into the softmax tail
    pr0 = mk_pr()
    po0 = mk_po()
    for jp in range(JP):
        av_pair(po0, jp, score_exp_pair(pr0, 0, jp))
    pr1 = mk_pr()
    pre = [score_exp_pair(pr1, 1, jp) for jp in range(KPRE)]
    pre += tail_and_y(pr0, po0, 0,
                      mid=lambda: [score_exp_pair(pr1, 1, jp)
                                   for jp in range(KPRE, KPRE + 2)])
    po1 = mk_po()
    for jp in range(JP):
        pT2 = pre[jp] if jp < len(pre) else score_exp_pair(pr1, 1, jp)
        av_pair(po1, jp, pT2)
    tail_and_y(pr1, po1, 1)


def build_module():
    nc = bacc.Bacc("TRN2", target_bir_lowering=False, debug=False,
                   num_devices=NCORES)
    d = {
        "x8p": nc.dram_tensor("x8p", [NP, P, 2, HW], FP8,
                              kind="ExternalInput").ap(),
        "x16q": nc.dram_tensor("x16q", [CT, P, QN], F16,
                               kind="ExternalInput").ap(),
        "wkbf": nc.dram_tensor("wkbf", [CT, P, C], BF16,
                               kind="ExternalInput").ap(),
        "wvbf": nc.dram_tensor("wvbf", [CT, P, C], BF16,
                               kind="ExternalInput").ap(),
        "wqbf": nc.dram_tensor("wqbf", [CT, P, C], BF16,
                               kind="ExternalInput").ap(),
        "wpT": nc.dram_tensor("wpT", [CT, P, C], F32R,
                              kind="ExternalInput").ap(),
        "cblob": nc.dram_tensor("cblob", [P, 32], F32,
                                kind="ExternalInput").ap(),
        "indT": nc.dram_tensor("indT", [GPT, P], F32,
                               kind="ExternalInput").ap(),
        "ones2": nc.dram_tensor("ones2", [P, 2, 16], FP8,
                                kind="ExternalInput").ap(),
        "y": nc.dram_tensor("y", [CT, P, QN], F32, kind="ExternalOutput").ap(),
    }
    with tile.TileContext(nc) as tc, ExitStack() as ctx:
        _build_body(nc, tc, ctx, d)
    nc.compile()
    return nc


_CACHE = {}


def _get_nc():
    if "nc" not in _CACHE:
        _CACHE["nc"] = build_module()
    return _CACHE["nc"]


def _shared_inputs(gamma, beta, wq, bq, wk, bk, wv, bv, wp, bp):
    bf16 = ml_dtypes.bfloat16
    f8 = ml_dtypes.float8_e4m3

    def wT8(w):
        wt = np.ascontiguousarray(np.asarray(w, np.float32).T) * 8.0
        return np.ascontiguousarray(wt.astype(bf16).reshape(CT, P, C))

    wpT = np.ascontiguousarray(
        np.asarray(wp, np.float32).T).reshape(CT, P, C)
    cblob = np.zeros((P, 32), np.float32)
    chv = np.stack([np.asarray(a, np.float32)
                    for a in (gamma, beta, bq, bk, bv, bp)],
                   axis=1).reshape(CT, P, 6)
    for t in range(CT):
        cblob[:, 6 * t:6 * t + 6] = chv[t]
    for i in range(P):
        cblob[i, 24 + i // 16] = 1.0 / 16.0
    indT = np.zeros((GPT, P), np.float32)
    for i in range(P):
        indT[i // 16, i] = 1.0
    return {
        "wkbf": wT8(wk), "wvbf": wT8(wv), "wqbf": wT8(wq),
        "wpT": np.ascontiguousarray(wpT),
        "cblob": cblob, "indT": indT,
        "ones2": np.ones((P, 2, 16), np.float32).astype(f8),
    }


def make_in_maps(x, gamma, beta, wq, bq, wk, bk, wv, bv, wp, bp):
    f8 = ml_dtypes.float8_e4m3
    shared = _shared_inputs(gamma, beta, wq, bq, wk, bk, wv, bv, wp, bp)
    xf = np.asarray(x, np.float32).reshape(B, C, HW)
    x8full = [xf[b].astype(f8) for b in range(B)]
    x16full = [xf[b].astype(np.float16) for b in range(B)]
    in_maps = []
    for core in range(NCORES):
        b, qc = divmod(core, NCORES // B)
        x8r = np.roll(x8full[b], -qc * QN, axis=1)       # [C, HW] fp8
        x8p = x8r.reshape(NP, 2, P, HW).transpose(0, 2, 1, 3)
        x16q = x16full[b][:, qc * QN:(qc + 1) * QN].reshape(CT, P, QN)
        m = dict(shared)
        m["x8p"] = np.ascontiguousarray(x8p)
        m["x16q"] = np.ascontiguousarray(x16q)
        in_maps.append(m)
    return in_maps


def assemble_output(results):
    out = np.empty((B, C, HW), np.float32)
    for core in range(NCORES):
        b, qc = divmod(core, NCORES // B)
        y = np.asarray(results[core]["y"]).reshape(C, QN)
        out[b, :, qc * QN:(qc + 1) * QN] = y
    return out.reshape(B, C, H, W)


def kernel(x, gamma, beta, wq, bq, wk, bk, wv, bv, wp, bp):
    nc = _get_nc()
    in_maps = make_in_maps(x, gamma, beta, wq, bq, wk, bk, wv, bv, wp, bp)
    res = run_bass_kernel_spmd(nc, in_maps, list(range(NCORES)))
    return assemble_output(res.results)
